# revision 7
# baseline (speedup 1.0000x reference)
"""CharRNN Trainium2 kernel.

Data-parallel over batch across 8 NeuronCores (16 batch rows per core,
small weights replicated). The sequential recurrence keeps the hidden
state transposed ("hT": hidden dim on partitions, packed columns
(step, kchunk, batch)) so the per-step matmuls consume and produce the
same layout with no per-step transposes.

Math per core (B=16 batch rows, H=512, V=E=128, L=1024):
  E2'[v, h]   = (embedding @ W_ih.T)[v, h] + b_h[h]          (setup)
  onehotT     = (x[c] == v)                                  (iota compare)
  xinT[h, c]  = (E2'.T @ onehotT)[h, c]                      (phase 1, -> HBM)
  hT(t+1)     = tanh(sum_k W_hh[j,k] @ hT_k(t) + xinT_t)     (phase 2, serial)
  logits      = hT.T @ W_ho.T + b_o                          (phase 3)

Everything bf16 on the PE except psum accumulation (fp32) and final
logits (fp32).
"""

import sys

sys.path.insert(0, "/opt/trn_rl_repo")

import numpy as np

from concourse import bass, tile, mybir
from concourse.bass_utils import run_bass_kernel_spmd

F32 = mybir.dt.float32
BF16 = mybir.dt.bfloat16
I16 = mybir.dt.int16

VOCAB = 128
EMBED = 128
HIDDEN = 512
BATCH = 128
SEQLEN = 1024
NCORES = 8
BPC = BATCH // NCORES  # 16 batch rows per core
NJ = HIDDEN // 128  # 4 hidden chunks

Tanh = mybir.ActivationFunctionType.Tanh
Alu = mybir.AluOpType


def split_multi_waits(nc):
    """This container's walrus supports one sync-wait per instruction; hoist
    extra waits into standalone EventSemaphore instructions just before."""
    n_split = 0
    for f in nc.m.functions:
        for b in f.blocks:
            new_instrs = []
            for ins in b.instructions:
                si = ins.sync_info
                waits = list(si.on_wait) if (si is not None and si.on_wait) else []
                if len(waits) > 1:
                    n_split += 1
                    for idx, w in enumerate(waits[:-1]):
                        ev = mybir.InstEventSemaphore(
                            name=f"{ins.name}-wsplit{idx}", ins=[], outs=[]
                        )
                        ev.engine = ins.engine
                        ev.sync_info = mybir.SyncInfo(on_wait=[w], on_update=[])
                        new_instrs.append(ev)
                    ins.sync_info = mybir.SyncInfo(
                        on_wait=[waits[-1]], on_update=list(si.on_update)
                    )
                new_instrs.append(ins)
            b.instructions = new_instrs
    return n_split


def build_nc(L=SEQLEN):
    C = BPC * L  # total (t, b) columns per core
    nc = bass.Bass(trn_type="TRN2")

    # ---- I/O ----
    xb_d = nc.declare_dram_parameter("xb", [128, C], I16, isOutput=False)
    hid_d = nc.declare_dram_parameter("hidden", [BPC, HIDDEN], F32, isOutput=False)
    emb_d = nc.declare_dram_parameter("embedding", [VOCAB, EMBED], F32, isOutput=False)
    wih_d = nc.declare_dram_parameter("W_ih", [HIDDEN, EMBED], F32, isOutput=False)
    whh_d = nc.declare_dram_parameter("W_hh", [HIDDEN, HIDDEN], F32, isOutput=False)
    bh_d = nc.declare_dram_parameter("b_h", [1, HIDDEN], F32, isOutput=False)
    who_d = nc.declare_dram_parameter("W_ho", [VOCAB, HIDDEN], F32, isOutput=False)
    bo_d = nc.declare_dram_parameter("b_o", [1, VOCAB], F32, isOutput=False)
    logits_d = nc.declare_dram_parameter("logits", [BPC, L, VOCAB], F32, isOutput=True)
    hout_d = nc.declare_dram_parameter("hidden_out", [BPC, HIDDEN], F32, isOutput=True)

    # internal scratch for the precomputed input projection (transposed)
    xinT_d = nc.dram_tensor("xinT_d", [NJ, 128, C], F32)

    with tile.TileContext(nc) as tc:
        with tc.tile_pool(name="const", bufs=1) as const:
            ones_s = const.tile([128, 128], F32, tag="ones")
            ident_s = const.tile([128, 128], F32, tag="ident")
            identb_s = const.tile([128, 128], BF16, tag="identb")
            iota_s = const.tile([128, 1], F32, tag="iota")
            wt_s = const.tile([128, 16 * 128], BF16, tag="wt")  # W_hh.T tiles (j,k)
            whoT_s = const.tile([128, HIDDEN], BF16, tag="whoT")  # W_ho.T
            e2_s = const.tile([128, HIDDEN], BF16, tag="e2")  # E2' bf16
            bh_s = const.tile([1, HIDDEN], F32, tag="bh")
            bo_s = const.tile([1, VOCAB], F32, tag="bo")

            nc.vector.memset(ones_s[:], 1.0)
            # identity = (p - c == 0) ? 1 : 0
            nc.gpsimd.affine_select(
                out=ident_s[:],
                in_=ones_s[:],
                pattern=[[-1, 128]],
                compare_op=Alu.is_equal,
                fill=0.0,
                base=0,
                channel_multiplier=1,
            )
            nc.vector.tensor_copy(identb_s[:], ident_s[:])
            nc.gpsimd.iota(
                iota_s[:],
                pattern=[[0, 1]],
                base=0,
                channel_multiplier=1,
                allow_small_or_imprecise_dtypes=True,
            )
            nc.sync.dma_start(bh_s[:], bh_d[:])
            nc.sync.dma_start(bo_s[:], bo_d[:])

            # ---- setup: load + transpose weights ----
            with (
                tc.tile_pool(name="setup", bufs=2) as setup,
                tc.tile_pool(name="pset", bufs=2, space="PSUM") as pset,
            ):
                # W_hh.T tiles: stationary for step MM (j out-chunk, k in-chunk)
                # wt[:, (j*4+k)*128:...] [kk, mm] = W_hh[j*128+mm, k*128+kk]
                for j in range(NJ):
                    whh_j = setup.tile([128, HIDDEN], F32, tag="whhj")
                    nc.sync.dma_start(whh_j[:], whh_d[j * 128 : (j + 1) * 128, :])
                    for k in range(NJ):
                        pt = pset.tile([128, 128], F32, tag="pt")
                        nc.tensor.transpose(
                            pt[:], whh_j[:, k * 128 : (k + 1) * 128], ident_s[:]
                        )
                        idx = j * 4 + k
                        nc.vector.tensor_copy(
                            wt_s[:, idx * 128 : (idx + 1) * 128], pt[:]
                        )
                # W_ho.T: whoT[:, k*128+v?] -> [hin, v] chunks along free dim? We
                # store as 4 chunks: whoT_s[:, k-chunk partition rows are hin of
                # chunk k] -- i.e. whoT_s[p, k*... ] layout: chunk k occupies
                # cols [k*128, (k+1)*128) with whoT_s[hh, k*128+v] = W_ho[v, k*128+hh]
                who_s = setup.tile([128, HIDDEN], F32, tag="whos")
                nc.sync.dma_start(who_s[:], who_d[:])
                for k in range(NJ):
                    pt = pset.tile([128, 128], F32, tag="pt")
                    nc.tensor.transpose(
                        pt[:], who_s[:, k * 128 : (k + 1) * 128], ident_s[:]
                    )
                    nc.vector.tensor_copy(whoT_s[:, k * 128 : (k + 1) * 128], pt[:])
                # embT, W_ihT -> E2' = emb @ W_ih.T + b_h   [v, h]
                emb_s = setup.tile([128, 128], F32, tag="embs")
                embT_s = setup.tile([128, 128], F32, tag="embT")
                wihT_s = setup.tile([128, HIDDEN], F32, tag="wihT")
                nc.sync.dma_start(emb_s[:], emb_d[:])
                pt = pset.tile([128, 128], F32, tag="pt")
                nc.tensor.transpose(pt[:], emb_s[:], ident_s[:])
                nc.vector.tensor_copy(embT_s[:], pt[:])
                for j in range(NJ):
                    wih_j = setup.tile([128, EMBED], F32, tag="wihj")
                    nc.sync.dma_start(wih_j[:], wih_d[j * 128 : (j + 1) * 128, :])
                    pt = pset.tile([128, 128], F32, tag="pt")
                    nc.tensor.transpose(pt[:], wih_j[:], ident_s[:])
                    nc.vector.tensor_copy(wihT_s[:, j * 128 : (j + 1) * 128], pt[:])
                pe2 = pset.tile([128, HIDDEN], F32, tag="pe2")
                nc.tensor.matmul(pe2[:], embT_s[:], wihT_s[:], start=True, stop=False)
                nc.tensor.matmul(
                    pe2[:], ones_s[0:1, :], bh_s[:], start=False, stop=True
                )
                nc.vector.tensor_copy(e2_s[:], pe2[:])

            # ---- phase 1: xinT = E2'.T @ onehot(x)  -> HBM (fp32) ----
            CHUNK = min(2048, C)
            with (
                tc.tile_pool(name="p1", bufs=2) as p1,
                tc.tile_pool(name="pp1", bufs=4, space="PSUM") as pp1,
            ):
                for c0 in range(0, C, CHUNK):
                    xb_c = p1.tile([128, CHUNK], I16, tag="xbc")
                    oh_c = p1.tile([128, CHUNK], BF16, tag="ohc")
                    nc.sync.dma_start(xb_c[:], xb_d[:, c0 : c0 + CHUNK])
                    nc.vector.tensor_scalar(
                        oh_c[:], xb_c[:], iota_s[:, 0:1], None, Alu.is_equal
                    )
                    for j in range(NJ):
                        for s0 in range(0, CHUNK, 512):
                            px = pp1.tile([128, 512], F32, tag="px")
                            nc.tensor.matmul(
                                px[:],
                                e2_s[:, j * 128 : (j + 1) * 128],
                                oh_c[:, s0 : s0 + 512],
                                start=True,
                                stop=True,
                            )
                            xs = p1.tile([128, 512], F32, tag="xs")
                            nc.vector.tensor_copy(xs[:], px[:])
                            nc.sync.dma_start(
                                xinT_d[j, :, c0 + s0 : c0 + s0 + 512], xs[:]
                            )

            # ---- hT storage: slot s holds h after s steps, packed cols
            # (local_t * 64 + k*16 + b).  Tile g covers slots 1+128g..128+128g.
            G = (L + 127) // 128
            ht0 = const.tile([128, 64], BF16, tag="ht0")  # slot 0
            htg = [
                const.tile(
                    [128, 64 * min(128, L - 128 * g)],
                    BF16,
                    tag=f"htg{g}",
                    name=f"htg{g}",
                )
                for g in range(G)
            ]

            def slot(s, k=None):
                """AP for slot s (cols k*16:(k+1)*16 if k given, else all 64)."""
                if s == 0:
                    t = ht0
                    col = 0
                else:
                    g = (s - 1) // 128
                    t = htg[g]
                    col = ((s - 1) % 128) * 64
                if k is None:
                    return t[:, col : col + 64]
                return t[:, col + k * 16 : col + (k + 1) * 16]

            # init slot 0 from the (transposed) initial hidden state
            with (
                tc.tile_pool(name="hinit", bufs=2) as hinit,
                tc.tile_pool(name="pinit", bufs=2, space="PSUM") as pinit,
            ):
                hid_s = hinit.tile([BPC, HIDDEN], F32, tag="hids")
                nc.sync.dma_start(hid_s[:], hid_d[:])
                for k in range(NJ):
                    pt = pinit.tile([128, BPC], F32, tag="pti")
                    nc.tensor.transpose(
                        pt[:], hid_s[:, k * 128 : (k + 1) * 128], ident_s[0:BPC, 0:BPC]
                    )
                    nc.vector.tensor_copy(slot(0, k), pt[:])

            # ---- phase 2: the recurrence ----
            SPS = 16  # steps per xin stage chunk
            with (
                tc.tile_pool(name="p2", bufs=3) as p2,
                tc.tile_pool(name="pp2", bufs=8, space="PSUM") as pp2,
            ):
                xstage = None
                for t in range(L):
                    if t % SPS == 0:
                        xstage = p2.tile([128, NJ, SPS * BPC], F32, tag="xst")
                        nc.sync.dma_start(
                            xstage[:],
                            xinT_d[:, :, t * BPC : (t + SPS) * BPC].rearrange(
                                "j p c -> p j c"
                            ),
                        )
                    lc = (t % SPS) * BPC
                    for j in range(NJ):
                        ph = pp2.tile([128, BPC], F32, tag="ph")
                        for k in range(NJ):
                            idx = j * 4 + k
                            nc.tensor.matmul(
                                ph[:],
                                wt_s[:, idx * 128 : (idx + 1) * 128],
                                slot(t, k),
                                start=(k == 0),
                                stop=(k == 3),
                            )
                        tmp = p2.tile([128, BPC], BF16, tag=f"tmp{j}")
                        nc.vector.scalar_tensor_tensor(
                            tmp[:],
                            ph[:],
                            1.0,
                            xstage[:, j, lc : lc + BPC],
                            Alu.mult,
                            Alu.add,
                        )
                        nc.scalar.activation(slot(t + 1, j), tmp[:], Tanh)

            # ---- phase 3: logits = hT.T @ W_ho.T + b_o ----
            with (
                tc.tile_pool(name="p3", bufs=4) as p3,
                tc.tile_pool(name="pp3", bufs=4, space="PSUM") as pp3,
            ):
                for g in range(G):
                    Mt = min(128, L - 128 * g)
                    hv = htg[g].rearrange("p (t j b) -> p t j b", j=NJ, b=BPC)
                    for b in range(BPC):
                        pl = pp3.tile([Mt, VOCAB], F32, tag="pl")
                        for k in range(NJ):
                            nc.tensor.matmul(
                                pl[:],
                                hv[:, 0:Mt, k, b],
                                whoT_s[:, k * 128 : (k + 1) * 128],
                                start=(k == 0),
                                stop=False,
                            )
                        nc.tensor.matmul(
                            pl[:], ones_s[0:1, 0:Mt], bo_s[:], start=False, stop=True
                        )
                        ls = p3.tile([Mt, VOCAB], F32, tag="ls")
                        nc.vector.tensor_copy(ls[:], pl[:])
                        nc.sync.dma_start(
                            logits_d[b, g * 128 : g * 128 + Mt, :], ls[:]
                        )

                # final hidden back to [b, h] fp32
                hstage = p3.tile([BPC, HIDDEN], F32, tag="hstage")
                for k in range(NJ):
                    pt = pp3.tile([BPC, 128], BF16, tag="ptf")
                    nc.tensor.transpose(pt[:], slot(L, k), identb_s[:])
                    nc.vector.tensor_copy(hstage[:, k * 128 : (k + 1) * 128], pt[:])
                nc.sync.dma_start(hout_d[:], hstage[:])

    split_multi_waits(nc)
    return nc


_cache = {}


def _get_nc(L):
    if L not in _cache:
        _cache[L] = build_nc(L)
    return _cache[L]


def kernel(x, hidden, embedding, W_ih, W_hh, b_h, W_ho, b_o, _L=None, _trace=False):
    x = np.asarray(x)
    hidden = np.asarray(hidden, dtype=np.float32)
    L = int(x.shape[1]) if _L is None else _L
    nc = _get_nc(L)

    weights = {
        "embedding": np.ascontiguousarray(embedding, dtype=np.float32),
        "W_ih": np.ascontiguousarray(W_ih, dtype=np.float32),
        "W_hh": np.ascontiguousarray(W_hh, dtype=np.float32),
        "b_h": np.ascontiguousarray(np.reshape(b_h, (1, HIDDEN)).astype(np.float32)),
        "W_ho": np.ascontiguousarray(W_ho, dtype=np.float32),
        "b_o": np.ascontiguousarray(np.reshape(b_o, (1, VOCAB)).astype(np.float32)),
    }
    in_maps = []
    for c in range(NCORES):
        xs = x[c * BPC : (c + 1) * BPC, :L].astype(np.int16)  # [16, L]
        xf = np.ascontiguousarray(xs.T).reshape(1, -1)  # t-major cols
        xb = np.ascontiguousarray(np.broadcast_to(xf, (128, BPC * L)))
        in_maps.append(
            {
                "xb": xb,
                "hidden": np.ascontiguousarray(hidden[c * BPC : (c + 1) * BPC]),
                **weights,
            }
        )

    res = run_bass_kernel_spmd(nc, in_maps, core_ids=list(range(NCORES)), trace=_trace)
    logits = np.concatenate([r["logits"] for r in res.results], axis=0)
    final_hidden = np.concatenate([r["hidden_out"] for r in res.results], axis=0)
    if _trace:
        kernel.last_exec_time_ns = res.exec_time_ns
        kernel.last_profile = res
    return logits, final_hidden


# revision 11
# speedup vs baseline: 5.3226x; 5.3226x over previous
"""CharRNN Trainium2 kernel.

Data-parallel over batch across 8 NeuronCores (16 batch rows per core,
small weights replicated). The sequential recurrence keeps the hidden
state transposed ("hT": hidden dim on partitions, packed columns
(step, kchunk, batch)) so the per-step matmuls consume and produce the
same layout with no per-step transposes.

Math per core (B=16 batch rows, H=512, V=E=128, L=1024):
  E2'[v, h]   = (embedding @ W_ih.T)[v, h] + b_h[h]          (setup)
  onehotT     = (x[c] == v)                                  (iota compare)
  xinT[h, c]  = (E2'.T @ onehotT)[h, c]                      (phase 1, -> HBM)
  hT(t+1)     = tanh(sum_k W_hh[j,k] @ hT_k(t) + xinT_t)     (phase 2, serial)
  logits      = hT.T @ W_ho.T + b_o                          (phase 3)

Everything bf16 on the PE except psum accumulation (fp32) and final
logits (fp32).
"""

import sys

sys.path.insert(0, "/opt/trn_rl_repo")

import numpy as np

from concourse import bass, tile, mybir
from concourse.bass_utils import run_bass_kernel_spmd

F32 = mybir.dt.float32
BF16 = mybir.dt.bfloat16
I16 = mybir.dt.int16

VOCAB = 128
EMBED = 128
HIDDEN = 512
BATCH = 128
SEQLEN = 1024
NCORES = 8
BPC = BATCH // NCORES  # 16 batch rows per core
NJ = HIDDEN // 128  # 4 hidden chunks

Tanh = mybir.ActivationFunctionType.Tanh
Alu = mybir.AluOpType


def split_multi_waits(nc):
    """This container's walrus supports one sync-wait per instruction; hoist
    extra waits into standalone EventSemaphore instructions just before."""
    n_split = 0
    for f in nc.m.functions:
        for b in f.blocks:
            new_instrs = []
            for ins in b.instructions:
                si = ins.sync_info
                waits = list(si.on_wait) if (si is not None and si.on_wait) else []
                if len(waits) > 1:
                    n_split += 1
                    for idx, w in enumerate(waits[:-1]):
                        ev = mybir.InstEventSemaphore(
                            name=f"{ins.name}-wsplit{idx}", ins=[], outs=[]
                        )
                        ev.engine = ins.engine
                        ev.sync_info = mybir.SyncInfo(on_wait=[w], on_update=[])
                        new_instrs.append(ev)
                    ins.sync_info = mybir.SyncInfo(
                        on_wait=[waits[-1]], on_update=list(si.on_update)
                    )
                new_instrs.append(ins)
            b.instructions = new_instrs
    return n_split


def build_nc(L=SEQLEN):
    C = BPC * L  # total (t, b) columns per core
    nc = bass.Bass(trn_type="TRN2")

    # ---- I/O ----
    xb_d = nc.declare_dram_parameter("xb", [128, C], I16, isOutput=False)
    hid_d = nc.declare_dram_parameter("hidden", [BPC, HIDDEN], F32, isOutput=False)
    emb_d = nc.declare_dram_parameter("embedding", [VOCAB, EMBED], F32, isOutput=False)
    wih_d = nc.declare_dram_parameter("W_ih", [HIDDEN, EMBED], F32, isOutput=False)
    whh_d = nc.declare_dram_parameter("W_hh", [HIDDEN, HIDDEN], F32, isOutput=False)
    bh_d = nc.declare_dram_parameter("b_h", [1, HIDDEN], F32, isOutput=False)
    who_d = nc.declare_dram_parameter("W_ho", [VOCAB, HIDDEN], F32, isOutput=False)
    bo_d = nc.declare_dram_parameter("b_o", [1, VOCAB], F32, isOutput=False)
    logits_d = nc.declare_dram_parameter("logits", [BPC, L, VOCAB], F32, isOutput=True)
    hout_d = nc.declare_dram_parameter("hidden_out", [BPC, HIDDEN], F32, isOutput=True)

    # internal scratch for the precomputed input projection (transposed)
    xinT_d = nc.dram_tensor("xinT_d", [NJ, 128, C], BF16)

    with tile.TileContext(nc) as tc:
        with tc.tile_pool(name="const", bufs=1) as const:
            ones_s = const.tile([128, 128], F32, tag="ones")
            ident_s = const.tile([128, 128], F32, tag="ident")
            identb_s = const.tile([128, 128], BF16, tag="identb")
            iota_s = const.tile([128, 1], F32, tag="iota")
            wt_s = const.tile([128, 16 * 128], BF16, tag="wt")  # W_hh.T tiles (j,k)
            whoT_s = const.tile([128, HIDDEN], BF16, tag="whoT")  # W_ho.T
            e2_s = const.tile([128, HIDDEN], BF16, tag="e2")  # E2' bf16
            bh_s = const.tile([1, HIDDEN], F32, tag="bh")
            bo_s = const.tile([1, VOCAB], F32, tag="bo")

            nc.vector.memset(ones_s[:], 1.0)
            # identity = (p - c == 0) ? 1 : 0
            nc.gpsimd.affine_select(
                out=ident_s[:],
                in_=ones_s[:],
                pattern=[[-1, 128]],
                compare_op=Alu.is_equal,
                fill=0.0,
                base=0,
                channel_multiplier=1,
            )
            nc.vector.tensor_copy(identb_s[:], ident_s[:])
            nc.gpsimd.iota(
                iota_s[:],
                pattern=[[0, 1]],
                base=0,
                channel_multiplier=1,
                allow_small_or_imprecise_dtypes=True,
            )
            nc.sync.dma_start(bh_s[:], bh_d[:])
            nc.sync.dma_start(bo_s[:], bo_d[:])

            # ---- setup: load + transpose weights ----
            with (
                tc.tile_pool(name="setup", bufs=2) as setup,
                tc.tile_pool(name="pset", bufs=2, space="PSUM") as pset,
            ):
                # W_hh.T tiles: stationary for step MM (j out-chunk, k in-chunk)
                # wt[:, (j*4+k)*128:...] [kk, mm] = W_hh[j*128+mm, k*128+kk]
                for j in range(NJ):
                    whh_j = setup.tile([128, HIDDEN], F32, tag="whhj")
                    nc.sync.dma_start(whh_j[:], whh_d[j * 128 : (j + 1) * 128, :])
                    for k in range(NJ):
                        pt = pset.tile([128, 128], F32, tag="pt")
                        nc.tensor.transpose(
                            pt[:], whh_j[:, k * 128 : (k + 1) * 128], ident_s[:]
                        )
                        idx = j * 4 + k
                        nc.vector.tensor_copy(
                            wt_s[:, idx * 128 : (idx + 1) * 128], pt[:]
                        )
                # W_ho.T: whoT[:, k*128+v?] -> [hin, v] chunks along free dim? We
                # store as 4 chunks: whoT_s[:, k-chunk partition rows are hin of
                # chunk k] -- i.e. whoT_s[p, k*... ] layout: chunk k occupies
                # cols [k*128, (k+1)*128) with whoT_s[hh, k*128+v] = W_ho[v, k*128+hh]
                who_s = setup.tile([128, HIDDEN], F32, tag="whos")
                nc.sync.dma_start(who_s[:], who_d[:])
                for k in range(NJ):
                    pt = pset.tile([128, 128], F32, tag="pt")
                    nc.tensor.transpose(
                        pt[:], who_s[:, k * 128 : (k + 1) * 128], ident_s[:]
                    )
                    nc.vector.tensor_copy(whoT_s[:, k * 128 : (k + 1) * 128], pt[:])
                # embT, W_ihT -> E2' = emb @ W_ih.T + b_h   [v, h]
                emb_s = setup.tile([128, 128], F32, tag="embs")
                embT_s = setup.tile([128, 128], F32, tag="embT")
                wihT_s = setup.tile([128, HIDDEN], F32, tag="wihT")
                nc.sync.dma_start(emb_s[:], emb_d[:])
                pt = pset.tile([128, 128], F32, tag="pt")
                nc.tensor.transpose(pt[:], emb_s[:], ident_s[:])
                nc.vector.tensor_copy(embT_s[:], pt[:])
                for j in range(NJ):
                    wih_j = setup.tile([128, EMBED], F32, tag="wihj")
                    nc.sync.dma_start(wih_j[:], wih_d[j * 128 : (j + 1) * 128, :])
                    pt = pset.tile([128, 128], F32, tag="pt")
                    nc.tensor.transpose(pt[:], wih_j[:], ident_s[:])
                    nc.vector.tensor_copy(wihT_s[:, j * 128 : (j + 1) * 128], pt[:])
                pe2 = pset.tile([128, HIDDEN], F32, tag="pe2")
                nc.tensor.matmul(pe2[:], embT_s[:], wihT_s[:], start=True, stop=False)
                nc.tensor.matmul(
                    pe2[:], ones_s[0:1, :], bh_s[:], start=False, stop=True
                )
                nc.vector.tensor_copy(e2_s[:], pe2[:])

            # ---- phase 1: xinT = E2'.T @ onehot(x)  -> HBM (fp32) ----
            CHUNK = min(2048, C)
            with (
                tc.tile_pool(name="p1", bufs=2) as p1,
                tc.tile_pool(name="pp1", bufs=4, space="PSUM") as pp1,
            ):
                for c0 in range(0, C, CHUNK):
                    xb_c = p1.tile([128, CHUNK], I16, tag="xbc")
                    oh_c = p1.tile([128, CHUNK], BF16, tag="ohc")
                    nc.sync.dma_start(xb_c[:], xb_d[:, c0 : c0 + CHUNK])
                    nc.vector.tensor_scalar(
                        oh_c[:], xb_c[:], iota_s[:, 0:1], None, Alu.is_equal
                    )
                    for j in range(NJ):
                        for s0 in range(0, CHUNK, 512):
                            px = pp1.tile([128, 512], F32, tag="px")
                            nc.tensor.matmul(
                                px[:],
                                e2_s[:, j * 128 : (j + 1) * 128],
                                oh_c[:, s0 : s0 + 512],
                                start=True,
                                stop=True,
                            )
                            xs = p1.tile([128, 512], BF16, tag="xs")
                            nc.vector.tensor_copy(xs[:], px[:])
                            nc.sync.dma_start(
                                xinT_d[j, :, c0 + s0 : c0 + s0 + 512], xs[:]
                            )

            # ---- hT storage: slot s holds h after s steps, packed cols
            # (local_t * 64 + k*16 + b).  Tile g covers slots 1+128g..128+128g.
            G = (L + 127) // 128
            ht0 = const.tile([128, 64], BF16, tag="ht0")  # slot 0
            htg = [
                const.tile(
                    [128, 64 * min(128, L - 128 * g)],
                    BF16,
                    tag=f"htg{g}",
                    name=f"htg{g}",
                )
                for g in range(G)
            ]

            def slot(s, k=None):
                """AP for slot s (cols k*16:(k+1)*16 if k given, else all 64)."""
                if s == 0:
                    t = ht0
                    col = 0
                else:
                    g = (s - 1) // 128
                    t = htg[g]
                    col = ((s - 1) % 128) * 64
                if k is None:
                    return t[:, col : col + 64]
                return t[:, col + k * 16 : col + (k + 1) * 16]

            # init slot 0 from the (transposed) initial hidden state
            with (
                tc.tile_pool(name="hinit", bufs=2) as hinit,
                tc.tile_pool(name="pinit", bufs=2, space="PSUM") as pinit,
            ):
                hid_s = hinit.tile([BPC, HIDDEN], F32, tag="hids")
                nc.sync.dma_start(hid_s[:], hid_d[:])
                for k in range(NJ):
                    pt = pinit.tile([128, BPC], F32, tag="pti")
                    nc.tensor.transpose(
                        pt[:], hid_s[:, k * 128 : (k + 1) * 128], ident_s[0:BPC, 0:BPC]
                    )
                    nc.vector.tensor_copy(slot(0, k), pt[:])

            # ---- phase 2: the recurrence ----
            # Per step: xin is injected into PSUM by an identity-matmul that
            # opens each accumulation group (start=True clears the bank and
            # writes xin), then 16 W_hh matmuls accumulate, then tanh reads
            # PSUM directly.  Two j-groups (hout 0:256 / 256:512) give two
            # contiguous [128, 32] tanh instructions per step so the tanh of
            # group 0 overlaps the matmuls of group 1 and of the next step.
            SPS = 16  # steps per xin stage chunk
            with (
                tc.tile_pool(name="p2", bufs=3) as p2,
                tc.tile_pool(name="pp2", bufs=4, space="PSUM") as pp2,
            ):
                xstage = None
                for t in range(L):
                    if t % SPS == 0:
                        xstage = p2.tile([128, NJ, SPS * BPC], BF16, tag="xst")
                        nc.sync.dma_start(
                            xstage[:],
                            xinT_d[:, :, t * BPC : (t + SPS) * BPC].rearrange(
                                "j p c -> p j c"
                            ),
                        )
                    lc = (t % SPS) * BPC
                    for g in range(2):
                        ph = pp2.tile([128, 2, BPC], F32, tag="ph")
                        phv = ph.rearrange("p a b -> p (a b)")
                        nc.tensor.matmul(
                            phv,
                            identb_s[:],
                            xstage[:, 2 * g : 2 * g + 2, lc : lc + BPC],
                            start=True,
                            stop=False,
                            skip_group_check=True,
                        )
                        for k in range(NJ):
                            for jj in range(2):
                                j = 2 * g + jj
                                idx = j * 4 + k
                                nc.tensor.matmul(
                                    ph[:, jj, :],
                                    wt_s[:, idx * 128 : (idx + 1) * 128],
                                    slot(t, k),
                                    start=False,
                                    stop=(k == 3 and jj == 1),
                                    skip_group_check=True,
                                )
                        nc.scalar.activation(
                            slot(t + 1)[:, g * 32 : (g + 1) * 32], phv, Tanh
                        )

            # ---- phase 3: logits = hT.T @ W_ho.T + b_o ----
            with (
                tc.tile_pool(name="p3", bufs=4) as p3,
                tc.tile_pool(name="pp3", bufs=4, space="PSUM") as pp3,
            ):
                for g in range(G):
                    Mt = min(128, L - 128 * g)
                    hv = htg[g].rearrange("p (t j b) -> p t j b", j=NJ, b=BPC)
                    for b in range(BPC):
                        pl = pp3.tile([Mt, VOCAB], F32, tag="pl")
                        for k in range(NJ):
                            nc.tensor.matmul(
                                pl[:],
                                hv[:, 0:Mt, k, b],
                                whoT_s[:, k * 128 : (k + 1) * 128],
                                start=(k == 0),
                                stop=False,
                            )
                        nc.tensor.matmul(
                            pl[:], ones_s[0:1, 0:Mt], bo_s[:], start=False, stop=True
                        )
                        ls = p3.tile([Mt, VOCAB], F32, tag="ls")
                        nc.vector.tensor_copy(ls[:], pl[:])
                        nc.sync.dma_start(
                            logits_d[b, g * 128 : g * 128 + Mt, :], ls[:]
                        )

                # final hidden back to [b, h] fp32
                hstage = p3.tile([BPC, HIDDEN], F32, tag="hstage")
                for k in range(NJ):
                    pt = pp3.tile([BPC, 128], BF16, tag="ptf")
                    nc.tensor.transpose(pt[:], slot(L, k), identb_s[:])
                    nc.vector.tensor_copy(hstage[:, k * 128 : (k + 1) * 128], pt[:])
                nc.sync.dma_start(hout_d[:], hstage[:])

    split_multi_waits(nc)
    return nc


_cache = {}


def _get_nc(L):
    if L not in _cache:
        _cache[L] = build_nc(L)
    return _cache[L]


def kernel(x, hidden, embedding, W_ih, W_hh, b_h, W_ho, b_o, _L=None, _trace=False):
    x = np.asarray(x)
    hidden = np.asarray(hidden, dtype=np.float32)
    L = int(x.shape[1]) if _L is None else _L
    nc = _get_nc(L)

    weights = {
        "embedding": np.ascontiguousarray(embedding, dtype=np.float32),
        "W_ih": np.ascontiguousarray(W_ih, dtype=np.float32),
        "W_hh": np.ascontiguousarray(W_hh, dtype=np.float32),
        "b_h": np.ascontiguousarray(np.reshape(b_h, (1, HIDDEN)).astype(np.float32)),
        "W_ho": np.ascontiguousarray(W_ho, dtype=np.float32),
        "b_o": np.ascontiguousarray(np.reshape(b_o, (1, VOCAB)).astype(np.float32)),
    }
    in_maps = []
    for c in range(NCORES):
        xs = x[c * BPC : (c + 1) * BPC, :L].astype(np.int16)  # [16, L]
        xf = np.ascontiguousarray(xs.T).reshape(1, -1)  # t-major cols
        xb = np.ascontiguousarray(np.broadcast_to(xf, (128, BPC * L)))
        in_maps.append(
            {
                "xb": xb,
                "hidden": np.ascontiguousarray(hidden[c * BPC : (c + 1) * BPC]),
                **weights,
            }
        )

    res = run_bass_kernel_spmd(nc, in_maps, core_ids=list(range(NCORES)), trace=_trace)
    logits = np.concatenate([r["logits"] for r in res.results], axis=0)
    final_hidden = np.concatenate([r["hidden_out"] for r in res.results], axis=0)
    if _trace:
        kernel.last_exec_time_ns = res.exec_time_ns
        kernel.last_profile = res
    return logits, final_hidden
